# revision 2
# baseline (speedup 1.0000x reference)
"""Causal multi-head attention (B=2, S=2048, D=1024, 16 heads x 64) on 8
Trainium2 NeuronCores.

Sharding: tensor-parallel over heads — 2 heads per core. Each core gets the
full (pre-transposed, bf16-cast) activations and its 2 heads' weights,
computes q/k/v projections, causal flash-style attention, and a partial
output projection; the host sums the 8 partial outputs and adds b_O.

Device algorithm per core (all matmuls bf16 with fp32 PSUM accumulate):
  - QKV:   qT/kT/vT [128=2*64 headdims, 4096 tok] = W.T @ xT, accumulated
           over 8 contraction chunks of 128. Host pre-arranges x and W into
           partition-major chunk-contiguous DRAM layouts so every DMA is a
           full-rate contiguous burst; late constants ride the gpsimd DMA
           queue so the sync queue only carries the critical x/w stream.
  - V is re-laid-out to [pos, headdim] via DVE 32x32 block transposes, with
    a ones-column appended so the attention-value matmul also produces the
    softmax denominator for free.
  - scores are computed transposed (key position on partitions) so softmax's
    sum folds into the AV matmul; the two heads' score matmuls run
    concurrently in disjoint PE row-groups. exp runs on the scalar engine
    straight out of PSUM. On diagonal key-tiles everything (scores, exp,
    mask, AV) is trimmed to the causally-needed query range; the causal
    mask multiply reduces to one shared 128x128 lower-triangle block.
  - QKV phase is interleaved with attention units per 1024-token block so
    the scalar engine's exp stream starts ~12us in instead of ~60us.
  - Q/K bias-add+cast run on the vector engine (tensor_scalar_add), V on
    the scalar engine: balances ACT vs DVE across the whole kernel.
  - 1/sum computed as exp(-ln(sum)) on the scalar engine, broadcast across
    partitions with a rank-2 matmul, applied while evacuating z.
  - out projection contracts both heads (128 partitions) in one matmul;
    its PSUM evacuation alternates engines at the tail.
"""

import functools

import numpy as np
import ml_dtypes

import concourse.bass as bass
import concourse.tile as tile
import concourse.mybir as mybir
from concourse.bass_utils import run_bass_kernel_spmd

# ---------------------------------------------------------------- wait fix
# This container's walrus accepts at most ONE sync-wait per instruction
# (two for EventSemaphore); Tile emits several. Hoist the excess onto NoOps
# inserted just before the over-subscribed instruction on the same engine.
import json as _json

_WAIT_CAP = {"EventSemaphore": 2}


def _split_waits(doc):
    n = [0]

    def fix_block(block):
        insts = block.get("instructions")
        if not isinstance(insts, list):
            return
        out = []
        for inst in insts:
            si = inst.get("sync_info")
            waits = si.get("on_wait") if si else None
            cap = _WAIT_CAP.get(inst.get("opcode"), 1)
            if waits and len(waits) > cap:
                for w in waits[cap:]:
                    n[0] += 1
                    out.append(
                        {
                            "name": f"WSPL-{n[0]}",
                            "opcode": "NoOp",
                            "engine": inst["engine"],
                            "ins": [],
                            "outs": [],
                            "sync_info": {"on_wait": [w], "on_update": []},
                        }
                    )
                si["on_wait"] = waits[:cap]
            out.append(inst)
        block["instructions"] = out

    def walk(o):
        if isinstance(o, dict):
            if "instructions" in o:
                fix_block(o)
            for v in o.values():
                walk(v)
        elif isinstance(o, list):
            for v in o:
                walk(v)

    walk(doc)
    return doc


_waitfix_done = False


def _install_waitfix():
    global _waitfix_done
    if _waitfix_done:
        return
    _waitfix_done = True
    orig = bass.Bass.to_json_bytes

    def to_json_bytes(self, *a, **kw):
        doc = _json.loads(orig(self, *a, **kw))
        return _json.dumps(_split_waits(doc)).encode()

    bass.Bass.to_json_bytes = to_json_bytes


# ---------------------------------------------------------------- constants
B, S, D = 2, 2048, 1024
NHEAD, HDIM = 16, 64
T = B * S  # 4096 tokens
NCORES = 8
HPC = NHEAD // NCORES  # 2 heads per core
SCALE = 1.0 / 8.0  # 1/sqrt(HDIM)

bf16 = mybir.dt.bfloat16
f32 = mybir.dt.float32
AF = mybir.ActivationFunctionType

NDC = D // 128  # 8 contraction chunks
NCHUNK = T // 512  # 8 token chunks of 512
NKT = S // 128  # 16 key tiles per batch
NQB = S // 512  # 4 query blocks per batch


def _build_nc():
    nc = bass.Bass()
    # chunk-major x: xh[p, k, a, m] = x[512k+m, 128a+p] — each 512-token
    # chunk is one contiguous 8KB-per-partition DMA
    xh = nc.dram_tensor("xh", [128, NCHUNK, NDC, 512], bf16, kind="ExternalInput")
    # group-major qkv weights: wh[p, g, a, c] = W_g[128a+p, c]
    wh = nc.dram_tensor("wh", [128, 3, NDC, 128], bf16, kind="ExternalInput")
    bqkv = nc.dram_tensor("bqkv", [128, 3], f32, kind="ExternalInput")
    wo = nc.dram_tensor("wo", [128, D], bf16, kind="ExternalInput")
    # single lower-triangle mask block (kk <= qq), replicated for 2 heads
    maskd = nc.dram_tensor("maskd", [128, HPC, 128], bf16, kind="ExternalInput")
    ones1 = nc.dram_tensor("ones1", [2, 128], bf16, kind="ExternalInput")
    outp = nc.dram_tensor("outp", [T, D], bf16, kind="ExternalOutput")

    with tile.TileContext(nc) as tc:
        with (
            tc.tile_pool(name="const", bufs=1) as const,
            tc.tile_pool(name="attn", bufs=8) as attnp,
            tc.tile_pool(name="obuf", bufs=4) as obufp,
            tc.tile_pool(name="small", bufs=4) as small,
            tc.tile_pool(name="psum", bufs=2, space="PSUM") as psum,
        ):
            # ---- constant loads. sync queue: V weights, then x chunks in
            # consumption order, then Q/K weights (needed a bit later).
            # gpsimd queue (idle engine): bias, ones, mask, wo.
            w_sb = const.tile([128, 3, NDC, 128], bf16)
            xt_sb = const.tile([128, NDC, T], bf16)
            nc.sync.dma_start(w_sb[:, 2], wh[:, 2])
            nc.sync.dma_start(xt_sb[:, :, 0:512], xh[:, 0])
            nc.sync.dma_start(xt_sb[:, :, 512:1024], xh[:, 1])
            nc.sync.dma_start(w_sb[:, 0], wh[:, 0])
            nc.sync.dma_start(w_sb[:, 1], wh[:, 1])

            bias_sb = const.tile([128, 3], f32)
            nc.gpsimd.dma_start(bias_sb[:], bqkv[:])
            ee_sb = const.tile([2, 128], bf16)
            nc.gpsimd.dma_start(ee_sb[:], ones1[:])
            mask_sb = const.tile([128, HPC, 128], bf16)
            nc.gpsimd.dma_start(mask_sb[:], maskd[:])
            wo_sb = const.tile([128, D], bf16)
            nc.gpsimd.dma_start(wo_sb[:], wo[:])

            for k in range(2, NCHUNK):
                sl = slice(512 * k, 512 * (k + 1))
                nc.sync.dma_start(xt_sb[:, :, sl], xh[:, k])

            qT = const.tile([128, T], bf16)
            kT = const.tile([128, T], bf16)
            vT = const.tile([128, T], bf16)
            zT = const.tile([128, T], bf16)
            qkvT = (qT, kT, vT)

            v_sb = []
            for h in range(HPC):
                v = const.tile([128, T // 128, 65], bf16, name=f"v_sb{h}")
                nc.gpsimd.memset(v[:, :, 64], 1.0)
                v_sb.append(v)

            # ---- QKV for one 1024-token block (V group first so the V
            # re-layout can start early). V cast on ACT (with bias), Q/K
            # casts on DVE tensor_scalar_add: balances engine load.
            vt4 = vT[:].rearrange("p (t x i) -> p t x i", x=4, i=32)

            def emit_qkv(pp):
                for g in (2, 0, 1):
                    ps = psum.tile([128, 1024], f32, tag="sc", bufs=3)
                    for half in range(2):
                        pt = 2 * pp + half
                        dst = ps[:, 512 * half : 512 * half + 512]
                        for di in range(NDC):
                            nc.tensor.matmul(
                                dst,
                                w_sb[:, g, di, :],
                                xt_sb[:, di, 512 * pt : 512 * pt + 512],
                                start=(di == 0),
                                stop=(di == NDC - 1),
                            )
                    dst_sb = qkvT[g][:, 1024 * pp : 1024 * pp + 1024]
                    if g == 2:
                        nc.scalar.activation(
                            dst_sb,
                            ps[:],
                            AF.Identity,
                            bias=bias_sb[:, g : g + 1],
                            scale=1.0,
                        )
                    else:
                        nc.vector.tensor_scalar_add(
                            dst_sb, ps[:], bias_sb[:, g : g + 1]
                        )
                # V block [1024*pp, 1024*(pp+1)) -> [pos, headdim] tiles
                ts = slice(8 * pp, 8 * pp + 8)
                for h in range(HPC):
                    for al in range(4):
                        for bb in range(2):
                            nc.vector.transpose(
                                v_sb[h][
                                    32 * al : 32 * al + 32, ts, 32 * bb : 32 * bb + 32
                                ],
                                vt4[
                                    64 * h + 32 * bb : 64 * h + 32 * bb + 32, ts, al, :
                                ],
                            )

            # ---- attention per (batch, 512-query-block). Each block's
            # output projection is emitted two units LATE so the in-order
            # PE always has ready matmuls while the previous block's
            # normalize chain completes.
            def emit_outproj(qb, b, tail=False):
                for qx in range(4):
                    qt = NKT * b + 4 * qb + qx
                    op = psum.tile([128, 1024], f32, tag="sc", bufs=3, name="op")
                    for dh in range(2):
                        nc.tensor.matmul(
                            op[:, 512 * dh : 512 * dh + 512],
                            zT[:, 128 * qt : 128 * qt + 128],
                            wo_sb[:, 512 * dh : 512 * dh + 512],
                            start=True,
                            stop=True,
                        )
                    ob = obufp.tile([128, 1024], bf16, name="ob")
                    if tail and qx % 2 == 1:
                        # drain: no exps left, the idle scalar engine
                        # shares the PSUM evacuation load with DVE
                        nc.scalar.copy(ob[:], op[:])
                    else:
                        nc.vector.tensor_copy(ob[:], op[:])
                    nc.gpsimd.dma_start(outp[128 * qt : 128 * qt + 128, :], ob[:])

            def norm_stage_a(st_):
                # 1/sum = exp(-ln(sum)); both heads' sums were DMA-staged
                # onto partitions {0,1} of one tile, so one ln and one exp
                # cover both heads
                q0, zsU, rsin, rs2 = st_
                lnS = small.tile([2, 512], f32, tag="lnS")
                nc.scalar.activation(lnS[:], rsin[:], AF.Ln, scale=1.0)
                nc.scalar.activation(rs2[:], lnS[:], AF.Exp, scale=-1.0)

            def norm_stage_b(st_):
                # broadcast both heads' reciprocals to 128 partitions with
                # one K=2 matmul against the 0/1 selector matrix ee_sb,
                # then normalize the staged z into zT
                q0, zsU, rsin, rs2 = st_
                rbP = psum.tile([128, 512], f32, tag="sc", bufs=3, name="rbP")
                nc.tensor.matmul(rbP[:], ee_sb[:], rs2[:], start=True, stop=True)
                for h in range(HPC):
                    nc.vector.tensor_mul(
                        zT[64 * h : 64 * h + 64, q0 : q0 + 512],
                        zsU[h][0:64, :],
                        rbP[64 * h : 64 * h + 64, :],
                    )

            st = {"norm_a": None, "norm_b": None, "uidx": 0}
            out_queue = []  # (uidx, qb, b); emitted two units late

            def emit_unit(qb, b):
                uidx = st["uidx"]
                st["uidx"] += 1
                norm_a = st["norm_a"]
                norm_b = st["norm_b"]
                nkt = 4 * (qb + 1)  # causal: key tiles 0..4qb+3
                q0 = S * b + 512 * qb
                zp = [
                    psum.tile([65, 512], f32, tag="z", bufs=2, name=f"zp{h}")
                    for h in range(HPC)
                ]
                for kt in range(nkt):
                    gk = NKT * b + kt
                    j = kt - 4 * qb  # >=0 on diagonal key-tiles
                    trim = 128 * j if j >= 0 else 0
                    sp = psum.tile([128, 2, 512], f32, tag="sc", bufs=3)
                    for h in range(HPC):
                        nc.tensor.matmul(
                            sp[:, h, trim:512],
                            kT[64 * h : 64 * h + 64, 128 * gk : 128 * gk + 128],
                            qT[64 * h : 64 * h + 64, q0 + trim : q0 + 512],
                            start=True,
                            stop=True,
                        )
                    at = attnp.tile([128, 2, 512], bf16)
                    nc.scalar.activation(
                        at[:, :, trim:512], sp[:, :, trim:512], AF.Exp, scale=SCALE
                    )
                    if j >= 0:
                        # causal mask: only the 128-wide diagonal block of
                        # the trimmed range can contain masked entries
                        nc.vector.tensor_mul(
                            at[:, :, trim : trim + 128],
                            at[:, :, trim : trim + 128],
                            mask_sb[:],
                        )
                    for h in range(HPC):
                        nc.tensor.matmul(
                            zp[h][:, trim:512],
                            v_sb[h][:, gk, :],
                            at[:, h, trim:512],
                            start=(kt == 0),
                            stop=(kt == nkt - 1),
                            skip_group_check=True,
                        )
                    if kt == 1 and norm_a is not None:
                        norm_stage_a(norm_a)
                        norm_b = norm_a
                        norm_a = None
                    if kt == min(4, nkt - 2) and norm_b is not None:
                        norm_stage_b(norm_b)
                        norm_b = None
                    if (
                        kt == min(5, nkt - 1)
                        and out_queue
                        and out_queue[0][0] <= uidx - 2
                    ):
                        _, oqb, ob_ = out_queue.pop(0)
                        emit_outproj(oqb, ob_)
                # evacuate z and its sums row to SBUF immediately so the
                # PSUM banks free up for the next query block; a small DMA
                # gathers the two sums rows onto partitions {0,1} of one
                # tile (DMA writes have no partition-alignment limits)
                zsU = [
                    small.tile([65, 512], bf16, tag=f"zsU{h}", name=f"zsU{h}")
                    for h in range(HPC)
                ]
                rsin = small.tile([2, 512], bf16, tag="rsin")
                rs2 = small.tile([2, 512], bf16, tag="rs2")
                for h in range(HPC):
                    nc.vector.tensor_copy(zsU[h][:], zp[h][:])
                    nc.sync.dma_start(rsin[h : h + 1, :], zsU[h][64:65, :])
                st["norm_a"] = (q0, zsU, rsin, rs2)
                st["norm_b"] = norm_b
                out_queue.append((uidx, qb, b))

            # ---- master schedule: QKV 1024-token blocks interleaved with
            # the attention units they unlock, so the scalar engine's exp
            # stream starts as soon as the first block's q/k/v exist and
            # QKV matmuls fill PE slack during scalar-bound attention.
            emit_qkv(0)
            emit_unit(0, 0)
            emit_unit(1, 0)
            emit_qkv(1)
            emit_unit(2, 0)
            emit_unit(3, 0)
            emit_qkv(2)
            emit_unit(0, 1)
            emit_unit(1, 1)
            emit_qkv(3)
            emit_unit(2, 1)
            emit_unit(3, 1)

            # tail: run the last unit's norm stages eagerly, interleaving
            # the two remaining output projections so the PE keeps busy
            # while the norm chain completes
            norm_stage_a(st["norm_a"])
            assert len(out_queue) == 2
            _, oqb, ob_ = out_queue.pop(0)
            emit_outproj(oqb, ob_, tail=True)
            norm_stage_b(st["norm_a"])
            _, oqb, ob_ = out_queue.pop(0)
            emit_outproj(oqb, ob_, tail=True)

    return nc


@functools.lru_cache(maxsize=1)
def _get_nc():
    _install_waitfix()
    return _build_nc()


def _to_bf16(a):
    return np.ascontiguousarray(np.asarray(a, dtype=np.float32)).astype(
        ml_dtypes.bfloat16
    )


def _prepare_in_maps(
    normalized_resid_pre, W_Q, W_K, W_V, W_O, b_Q, b_K, b_V, b_O
):
    x = np.asarray(normalized_resid_pre, dtype=np.float32)
    W_Q = np.asarray(W_Q, dtype=np.float32)
    W_K = np.asarray(W_K, dtype=np.float32)
    W_V = np.asarray(W_V, dtype=np.float32)
    W_O = np.asarray(W_O, dtype=np.float32)
    b_Q = np.asarray(b_Q, dtype=np.float32)
    b_K = np.asarray(b_K, dtype=np.float32)
    b_V = np.asarray(b_V, dtype=np.float32)
    b_O = np.asarray(b_O, dtype=np.float32)

    # xh[p, k, a, m] = x[512k+m, 128a+p]
    xh = _to_bf16(
        x.reshape(T, D).reshape(NCHUNK, 512, NDC, 128).transpose(3, 0, 2, 1)
    )

    # shared lower-triangle diagonal mask block (kk <= qq), both heads
    kk = np.arange(128)[:, None]
    qq = np.arange(128)[None, :]
    maskd = np.broadcast_to(
        (kk <= qq).astype(np.float32)[:, None, :], (128, HPC, 128)
    )
    maskd = np.ascontiguousarray(maskd).astype(ml_dtypes.bfloat16)

    ones_np = np.zeros((2, 128), np.float32)
    ones_np[0, :64] = 1.0
    ones_np[1, 64:] = 1.0
    ones_np = ones_np.astype(ml_dtypes.bfloat16)

    in_maps = []
    for c in range(NCORES):
        h0, h1 = HPC * c, HPC * c + 1
        # wh[p, g, a, c] = W_g[128a+p, c] with W_g = 2 heads side by side
        wh_c = np.stack(
            [
                np.concatenate([W_Q[h0], W_Q[h1]], axis=1),
                np.concatenate([W_K[h0], W_K[h1]], axis=1),
                np.concatenate([W_V[h0], W_V[h1]], axis=1),
            ]
        )  # [3, 1024, 128]
        wh_c = wh_c.reshape(3, NDC, 128, 128).transpose(2, 0, 1, 3)
        bqkv_c = np.stack(
            [
                np.concatenate([b_Q[h0], b_Q[h1]]),
                np.concatenate([b_K[h0], b_K[h1]]),
                np.concatenate([b_V[h0], b_V[h1]]),
            ],
            axis=1,
        ).astype(np.float32)
        wo_c = np.concatenate([W_O[h0], W_O[h1]], axis=0)
        in_maps.append(
            {
                "xh": xh,
                "wh": _to_bf16(wh_c),
                "bqkv": np.ascontiguousarray(bqkv_c),
                "wo": _to_bf16(wo_c),
                "maskd": maskd,
                "ones1": ones_np,
            }
        )
    return in_maps, b_O


def _gather(res, b_O):
    out = np.zeros((T, D), np.float32)
    for r in res.results:
        out += r["outp"].astype(np.float32)
    out += b_O[None, :]
    return out.reshape(B, S, D)


def kernel(
    normalized_resid_pre, W_Q, W_K, W_V, W_O, b_Q, b_K, b_V, b_O, **_unused
):
    in_maps, b_O = _prepare_in_maps(
        normalized_resid_pre, W_Q, W_K, W_V, W_O, b_Q, b_K, b_V, b_O
    )
    nc = _get_nc()
    res = run_bass_kernel_spmd(nc, in_maps, core_ids=list(range(NCORES)))
    return _gather(res, b_O)


def _try_install_profhook():
    """Register the axon NTFF profile hook (the container's antenv stub
    lacks axon_hooks); harmless no-op if anything is missing."""
    try:
        import sys
        import types

        if "antenv.axon_hooks" not in sys.modules:
            mod = types.ModuleType("antenv.axon_hooks")
            hook = [None]
            mod.set_axon_ntff_profile_hook = lambda h: hook.__setitem__(0, h)
            mod.get_axon_ntff_profile_hook = lambda: hook[0]
            sys.modules["antenv.axon_hooks"] = mod
            import antenv

            antenv.axon_hooks = mod
            from trn_agent_boot.trn_boot import _ntff_profile_via_ctypes

            mod.set_axon_ntff_profile_hook(
                _ntff_profile_via_ctypes("/opt/axon/libaxon_pjrt.so")
            )
            import concourse.bass_utils as bu

            bu.upload_artifacts = lambda tmpdir: f"file://{tmpdir}"
    except Exception:
        pass


def kernel_profiled(**inputs):
    """Like kernel() but with NTFF tracing; returns (out, BassKernelResults)."""
    _try_install_profhook()
    inputs = {k: v for k, v in inputs.items()}
    in_maps, b_O = _prepare_in_maps(
        inputs["normalized_resid_pre"],
        inputs["W_Q"],
        inputs["W_K"],
        inputs["W_V"],
        inputs["W_O"],
        inputs["b_Q"],
        inputs["b_K"],
        inputs["b_V"],
        inputs["b_O"],
    )
    nc = _get_nc()
    res = run_bass_kernel_spmd(
        nc, in_maps, core_ids=list(range(NCORES)), trace=True
    )
    return _gather(res, b_O), res


if __name__ == "__main__":
    rng = np.random.default_rng(0)
    inputs = {
        "normalized_resid_pre": rng.standard_normal((B, S, D)).astype(np.float32),
        "W_Q": (rng.standard_normal((NHEAD, D, HDIM)) * 0.02).astype(np.float32),
        "W_K": (rng.standard_normal((NHEAD, D, HDIM)) * 0.02).astype(np.float32),
        "W_V": (rng.standard_normal((NHEAD, D, HDIM)) * 0.02).astype(np.float32),
        "W_O": (rng.standard_normal((NHEAD, HDIM, D)) * 0.02).astype(np.float32),
        "b_Q": np.zeros((NHEAD, HDIM), np.float32),
        "b_K": np.zeros((NHEAD, HDIM), np.float32),
        "b_V": np.zeros((NHEAD, HDIM), np.float32),
        "b_O": np.zeros((D,), np.float32),
    }
    out = kernel(**inputs)
    print("out", out.shape, out.dtype, float(np.abs(out).max()))


# revision 10
# speedup vs baseline: 1.2097x; 1.2097x over previous
"""Causal multi-head attention (B=2, S=2048, D=1024, 16 heads x 64) on 8
Trainium2 NeuronCores.

Sharding: tensor-parallel over heads — 2 heads per core. Each core gets the
full (pre-transposed, bf16-cast) activations and its 2 heads' weights,
computes q/k/v projections, causal flash-style attention, and a partial
output projection; the host sums the 8 partial outputs and adds b_O.

Device algorithm per core (all matmuls bf16 with fp32 PSUM accumulate):
  - QKV:   qT/kT/vT [128=2*64 headdims, 4096 tok] = W.T @ xT, accumulated
           over 8 contraction chunks of 128. Host pre-arranges x and W into
           partition-major chunk-contiguous DRAM layouts so every DMA is a
           full-rate contiguous burst; late constants ride the gpsimd DMA
           queue so the sync queue only carries the critical x/w stream.
  - V is re-laid-out to [pos, headdim] via DVE 32x32 block transposes, with
    a ones-column appended so the attention-value matmul also produces the
    softmax denominator for free.
  - scores are computed transposed (key position on partitions) so softmax's
    sum folds into the AV matmul; the two heads' score matmuls run
    concurrently in disjoint PE row-groups. exp runs on the scalar engine
    straight out of PSUM. On diagonal key-tiles everything (scores, exp,
    mask, AV) is trimmed to the causally-needed query range; the causal
    mask multiply reduces to one shared 128x128 lower-triangle block.
  - QKV phase is interleaved with attention units per 1024-token block so
    the scalar engine's exp stream starts ~12us in instead of ~60us.
  - Q/K bias-add+cast run on the vector engine (tensor_scalar_add), V on
    the scalar engine: balances ACT vs DVE across the whole kernel.
  - 1/sum computed as exp(-ln(sum)) on the scalar engine, broadcast across
    partitions with a rank-2 matmul, applied while evacuating z.
  - out projection contracts both heads (128 partitions) in one matmul;
    its PSUM evacuation alternates engines at the tail.
"""

import functools

import numpy as np
import ml_dtypes

import concourse.bass as bass
import concourse.tile as tile
import concourse.mybir as mybir
from concourse.bass_utils import run_bass_kernel_spmd

# ---------------------------------------------------------------- wait fix
# This container's walrus accepts at most ONE sync-wait per instruction
# (two for EventSemaphore); Tile emits several. Hoist the excess onto NoOps
# inserted just before the over-subscribed instruction on the same engine.
import json as _json

_WAIT_CAP = {"EventSemaphore": 2}


def _split_waits(doc):
    n = [0]

    def fix_block(block):
        insts = block.get("instructions")
        if not isinstance(insts, list):
            return
        out = []
        for inst in insts:
            si = inst.get("sync_info")
            waits = si.get("on_wait") if si else None
            cap = _WAIT_CAP.get(inst.get("opcode"), 1)
            if waits and len(waits) > cap:
                for w in waits[cap:]:
                    n[0] += 1
                    out.append(
                        {
                            "name": f"WSPL-{n[0]}",
                            "opcode": "NoOp",
                            "engine": inst["engine"],
                            "ins": [],
                            "outs": [],
                            "sync_info": {"on_wait": [w], "on_update": []},
                        }
                    )
                si["on_wait"] = waits[:cap]
            out.append(inst)
        block["instructions"] = out

    def walk(o):
        if isinstance(o, dict):
            if "instructions" in o:
                fix_block(o)
            for v in o.values():
                walk(v)
        elif isinstance(o, list):
            for v in o:
                walk(v)

    walk(doc)
    return doc


_waitfix_done = False


def _install_waitfix():
    global _waitfix_done
    if _waitfix_done:
        return
    _waitfix_done = True
    orig = bass.Bass.to_json_bytes

    def to_json_bytes(self, *a, **kw):
        doc = _json.loads(orig(self, *a, **kw))
        return _json.dumps(_split_waits(doc)).encode()

    bass.Bass.to_json_bytes = to_json_bytes


# ---------------------------------------------------------------- constants
B, S, D = 2, 2048, 1024
NHEAD, HDIM = 16, 64
T = B * S  # 4096 tokens
NCORES = 8
HPC = NHEAD // NCORES  # 2 heads per core
SCALE = 1.0 / 8.0  # 1/sqrt(HDIM)

bf16 = mybir.dt.bfloat16
f32 = mybir.dt.float32
AF = mybir.ActivationFunctionType

NDC = D // 128  # 8 contraction chunks
NCHUNK = T // 512  # 8 token chunks of 512
NKT = S // 128  # 16 key tiles per batch
NQB = S // 512  # 4 query blocks per batch


def _build_nc():
    nc = bass.Bass()
    # chunk-major x: xh[p, k, a, m] = x[512k+m, 128a+p] — each 512-token
    # chunk is one contiguous 8KB-per-partition DMA into the identically
    # laid-out SBUF tile
    xh = nc.dram_tensor("xh", [128, NCHUNK, NDC, 512], bf16, kind="ExternalInput")
    # group-major qkv weights: wh[p, g, a, c] = W_g[128a+p, c]
    wh = nc.dram_tensor("wh", [128, 3, NDC, 128], bf16, kind="ExternalInput")
    bqkv = nc.dram_tensor("bqkv", [128, 3], f32, kind="ExternalInput")
    wo = nc.dram_tensor("wo", [128, D], bf16, kind="ExternalInput")
    # single lower-triangle mask block (kk <= qq), replicated for 2 heads
    maskd = nc.dram_tensor("maskd", [128, HPC, 128], bf16, kind="ExternalInput")
    ones1 = nc.dram_tensor("ones1", [2, 128], bf16, kind="ExternalInput")
    outp = nc.dram_tensor("outp", [T, D], bf16, kind="ExternalOutput")

    with tile.TileContext(nc) as tc:
        with (
            tc.tile_pool(name="const", bufs=1) as const,
            tc.tile_pool(name="attn", bufs=8) as attnp,
            tc.tile_pool(name="obuf", bufs=4) as obufp,
            tc.tile_pool(name="small", bufs=4) as small,
            tc.tile_pool(name="psum", bufs=2, space="PSUM") as psum,
        ):
            # ---- constant loads. sync queue: V weights, then x chunks in
            # consumption order (chunk 0 split in half so the first matmul
            # group can start ~1.5us earlier), then Q/K weights.
            # gpsimd queue (idle engine): bias, ones, mask, wo.
            w_sb = const.tile([128, 3, NDC, 128], bf16)
            xt_sb = const.tile([128, NCHUNK, NDC, 512], bf16)
            nc.sync.dma_start(w_sb[:, 2], wh[:, 2])
            nc.sync.dma_start(xt_sb[:, 0, :, 0:256], xh[:, 0, :, 0:256])
            nc.sync.dma_start(xt_sb[:, 0, :, 256:512], xh[:, 0, :, 256:512])
            nc.sync.dma_start(xt_sb[:, 1], xh[:, 1])
            nc.sync.dma_start(w_sb[:, 0], wh[:, 0])
            nc.sync.dma_start(w_sb[:, 1], wh[:, 1])

            bias_sb = const.tile([128, 3], f32)
            nc.gpsimd.dma_start(bias_sb[:], bqkv[:])
            ee_sb = const.tile([2, 128], bf16)
            nc.gpsimd.dma_start(ee_sb[:], ones1[:])
            mask_sb = const.tile([128, HPC, 128], bf16)
            nc.gpsimd.dma_start(mask_sb[:], maskd[:])
            wo_sb = const.tile([128, D], bf16)
            nc.gpsimd.dma_start(wo_sb[:], wo[:])

            for k in range(2, NCHUNK):
                nc.sync.dma_start(xt_sb[:, k], xh[:, k])

            qT = const.tile([128, T], bf16)
            kT = const.tile([128, T], bf16)
            vT = const.tile([128, T], bf16)
            zT = const.tile([128, T], bf16)
            qkvT = (qT, kT, vT)

            v_sb = []
            for h in range(HPC):
                v = const.tile([128, T // 128, 65], bf16, name=f"v_sb{h}")
                nc.gpsimd.memset(v[:, :, 64], 1.0)
                v_sb.append(v)

            # ---- QKV for one 512-token half-block or a full 1024-token
            # block (V group first so the V re-layout can start early).
            # V cast on ACT (with bias), Q/K casts on DVE tensor_scalar_add:
            # balances ACT vs DVE load across the kernel.
            vt4 = vT[:].rearrange("p (t x i) -> p t x i", x=4, i=32)

            def emit_qkv(pp, halves=(0, 1)):
                nh = len(halves)
                for g in (2, 0, 1):
                    ps = psum.tile([128, 512 * nh], f32, tag="sc", bufs=3)
                    for i, half in enumerate(halves):
                        pt = 2 * pp + half
                        dst = ps[:, 512 * i : 512 * i + 512]
                        for di in range(NDC):
                            nc.tensor.matmul(
                                dst,
                                w_sb[:, g, di, :],
                                xt_sb[:, pt, di, :],
                                start=(di == 0),
                                stop=(di == NDC - 1),
                            )
                    c0 = 1024 * pp + 512 * halves[0]
                    dst_sb = qkvT[g][:, c0 : c0 + 512 * nh]
                    if g == 2:
                        nc.scalar.activation(
                            dst_sb,
                            ps[:],
                            AF.Identity,
                            bias=bias_sb[:, g : g + 1],
                            scale=1.0,
                        )
                    else:
                        nc.vector.tensor_scalar_add(
                            dst_sb, ps[:], bias_sb[:, g : g + 1]
                        )
                # V tokens -> [pos, headdim] tiles
                ts = slice(8 * pp + 4 * halves[0], 8 * pp + 4 * halves[0] + 4 * nh)
                for h in range(HPC):
                    for al in range(4):
                        for bb in range(2):
                            nc.vector.transpose(
                                v_sb[h][
                                    32 * al : 32 * al + 32, ts, 32 * bb : 32 * bb + 32
                                ],
                                vt4[
                                    64 * h + 32 * bb : 64 * h + 32 * bb + 32, ts, al, :
                                ],
                            )

            # ---- attention per (batch, 512-query-block). Each block's
            # output projection is emitted two units LATE so the in-order
            # PE always has ready matmuls while the previous block's
            # normalize chain completes.
            def emit_outproj_tile(qb, b, qx, tail=False):
                qt = NKT * b + 4 * qb + qx
                op = psum.tile([128, 1024], f32, tag="sc", bufs=3, name="op")
                for dh in range(2):
                    nc.tensor.matmul(
                        op[:, 512 * dh : 512 * dh + 512],
                        zT[:, 128 * qt : 128 * qt + 128],
                        wo_sb[:, 512 * dh : 512 * dh + 512],
                        start=True,
                        stop=True,
                    )
                ob = obufp.tile([128, 1024], bf16, name="ob")
                if tail and qx % 2 == 1:
                    # drain: no exps left, the idle scalar engine
                    # shares the PSUM evacuation load with DVE
                    nc.scalar.copy(ob[:], op[:])
                else:
                    nc.vector.tensor_copy(ob[:], op[:])
                nc.sync.dma_start(outp[128 * qt : 128 * qt + 128, :], ob[:])

            def emit_outproj(qb, b, tail=False):
                for qx in range(4):
                    emit_outproj_tile(qb, b, qx, tail=tail)

            def norm_stage_a(st_):
                # 1/sum = exp(-ln(sum)); both heads' sums were DMA-staged
                # onto partitions {0,1} of one tile, so one ln and one exp
                # cover both heads
                q0, zsU, rsin, rs2 = st_
                lnS = small.tile([2, 512], f32, tag="lnS")
                nc.scalar.activation(lnS[:], rsin[:], AF.Ln, scale=1.0)
                nc.scalar.activation(rs2[:], lnS[:], AF.Exp, scale=-1.0)

            def norm_stage_b(st_):
                # broadcast both heads' reciprocals to 128 partitions with
                # one K=2 matmul against the 0/1 selector matrix ee_sb,
                # then normalize the staged z into zT
                q0, zsU, rsin, rs2 = st_
                rbP = psum.tile([128, 512], f32, tag="sc", bufs=3, name="rbP")
                nc.tensor.matmul(rbP[:], ee_sb[:], rs2[:], start=True, stop=True)
                for h in range(HPC):
                    nc.vector.tensor_mul(
                        zT[64 * h : 64 * h + 64, q0 : q0 + 512],
                        zsU[h][0:64, :],
                        rbP[64 * h : 64 * h + 64, :],
                    )

            st = {"norm_a": None, "norm_b": None, "uidx": 0}
            out_queue = []  # (uidx, qb, b); emitted two units late

            def emit_unit(qb, b):
                uidx = st["uidx"]
                st["uidx"] += 1
                norm_a = st["norm_a"]
                norm_b = st["norm_b"]
                nkt = 4 * (qb + 1)  # causal: key tiles 0..4qb+3
                q0 = S * b + 512 * qb
                zp = [
                    psum.tile([65, 512], f32, tag="z", bufs=2, name=f"zp{h}")
                    for h in range(HPC)
                ]
                # pop one deferred output-projection block and spread its 4
                # tiles across this unit's key-tile iterations so each op
                # PSUM tile is evacuated before the next is written
                op_todo = []
                if out_queue and out_queue[0][0] <= uidx - 2:
                    _, oqb, ob_ = out_queue.pop(0)
                    op_kts = [1, 2, 3, 3] if nkt == 4 else [2, 3, 5, 6]
                    op_todo = [(k, oqb, ob_, qx) for qx, k in enumerate(op_kts)]

                # software pipeline: AV(kt) is emitted during iteration
                # kt+1, so the in-order PE streams scores(kt+1) while the
                # scalar engine runs exp(kt); AV never stalls the chain.
                pend = None  # (at, gk, trim, is_last)

                def emit_av(p):
                    at_, gk_, trim_, last_ = p
                    for h in range(HPC):
                        nc.tensor.matmul(
                            zp[h][:, trim_:512],
                            v_sb[h][:, gk_, :],
                            at_[:, h, trim_:512],
                            start=(gk_ % NKT == 0),
                            stop=last_,
                            skip_group_check=True,
                        )

                for kt in range(nkt):
                    gk = NKT * b + kt
                    j = kt - 4 * qb  # >=0 on diagonal key-tiles
                    trim = 128 * j if j >= 0 else 0
                    # flat [128, 1024] tiles: a 2D AP is ~200ns/instruction
                    # cheaper on ACT than the equivalent 3D view; 3D views
                    # are used only for the trimmed diagonal slices
                    sp = psum.tile([128, 1024], f32, tag="sc", bufs=3)
                    for h in range(HPC):
                        nc.tensor.matmul(
                            sp[:, 512 * h + trim : 512 * h + 512],
                            kT[64 * h : 64 * h + 64, 128 * gk : 128 * gk + 128],
                            qT[64 * h : 64 * h + 64, q0 + trim : q0 + 512],
                            start=True,
                            stop=True,
                        )
                    at = attnp.tile([128, 1024], bf16)
                    if j >= 0:
                        sp3 = sp[:].rearrange("p (h q) -> p h q", h=2)
                        at3 = at[:].rearrange("p (h q) -> p h q", h=2)
                        nc.scalar.activation(
                            at3[:, :, trim:512],
                            sp3[:, :, trim:512],
                            AF.Exp,
                            scale=SCALE,
                        )
                        # causal mask: only the 128-wide diagonal block of
                        # the trimmed range can contain masked entries
                        nc.vector.tensor_mul(
                            at3[:, :, trim : trim + 128],
                            at3[:, :, trim : trim + 128],
                            mask_sb[:],
                        )
                    else:
                        nc.scalar.activation(at[:], sp[:], AF.Exp, scale=SCALE)
                    if pend is not None:
                        emit_av(pend)
                    pend = (at, gk, trim, kt == nkt - 1)
                    if kt == 1 and norm_a is not None:
                        norm_stage_a(norm_a)
                        norm_b = norm_a
                        norm_a = None
                    if kt == min(4, nkt - 2) and norm_b is not None:
                        norm_stage_b(norm_b)
                        norm_b = None
                    while op_todo and op_todo[0][0] == kt:
                        _, oqb, ob_, qx = op_todo.pop(0)
                        emit_outproj_tile(oqb, ob_, qx)
                emit_av(pend)
                # evacuate z and its sums row to SBUF immediately so the
                # PSUM banks free up for the next query block; a small DMA
                # gathers the two sums rows onto partitions {0,1} of one
                # tile (DMA writes have no partition-alignment limits)
                zsU = [
                    small.tile([65, 512], bf16, tag=f"zsU{h}", name=f"zsU{h}")
                    for h in range(HPC)
                ]
                rsin = small.tile([2, 512], bf16, tag="rsin")
                rs2 = small.tile([2, 512], bf16, tag="rs2")
                for h in range(HPC):
                    nc.vector.tensor_copy(zsU[h][:], zp[h][:])
                    nc.sync.dma_start(rsin[h : h + 1, :], zsU[h][64:65, :])
                st["norm_a"] = (q0, zsU, rsin, rs2)
                st["norm_b"] = norm_b
                out_queue.append((uidx, qb, b))

            # ---- master schedule: QKV blocks interleaved with the
            # attention units they unlock, so the scalar engine's exp
            # stream starts as soon as the first 512 tokens' q/k/v exist
            # and QKV matmuls fill PE slack during scalar-bound attention.
            emit_qkv(0, halves=(0,))
            emit_unit(0, 0)
            emit_qkv(0, halves=(1,))
            emit_unit(1, 0)
            emit_qkv(1)
            emit_unit(2, 0)
            emit_unit(3, 0)
            emit_qkv(2)
            emit_unit(0, 1)
            emit_unit(1, 1)
            emit_qkv(3)
            emit_unit(2, 1)
            emit_unit(3, 1)

            # tail: run the last unit's norm stages eagerly, interleaving
            # the two remaining output projections so the PE keeps busy
            # while the norm chain completes
            norm_stage_a(st["norm_a"])
            assert len(out_queue) == 2
            _, oqb, ob_ = out_queue.pop(0)
            emit_outproj(oqb, ob_, tail=True)
            norm_stage_b(st["norm_a"])
            _, oqb, ob_ = out_queue.pop(0)
            emit_outproj(oqb, ob_, tail=True)

    return nc


@functools.lru_cache(maxsize=1)
def _get_nc():
    _install_waitfix()
    return _build_nc()


def _to_bf16(a):
    return np.ascontiguousarray(np.asarray(a, dtype=np.float32)).astype(
        ml_dtypes.bfloat16
    )


def _prepare_in_maps(
    normalized_resid_pre, W_Q, W_K, W_V, W_O, b_Q, b_K, b_V, b_O
):
    x = np.asarray(normalized_resid_pre, dtype=np.float32)
    W_Q = np.asarray(W_Q, dtype=np.float32)
    W_K = np.asarray(W_K, dtype=np.float32)
    W_V = np.asarray(W_V, dtype=np.float32)
    W_O = np.asarray(W_O, dtype=np.float32)
    b_Q = np.asarray(b_Q, dtype=np.float32)
    b_K = np.asarray(b_K, dtype=np.float32)
    b_V = np.asarray(b_V, dtype=np.float32)
    b_O = np.asarray(b_O, dtype=np.float32)

    # xh[p, k, a, m] = x[512k+m, 128a+p]
    xh = _to_bf16(
        x.reshape(T, D).reshape(NCHUNK, 512, NDC, 128).transpose(3, 0, 2, 1)
    )

    # shared lower-triangle diagonal mask block (kk <= qq), both heads
    kk = np.arange(128)[:, None]
    qq = np.arange(128)[None, :]
    maskd = np.broadcast_to(
        (kk <= qq).astype(np.float32)[:, None, :], (128, HPC, 128)
    )
    maskd = np.ascontiguousarray(maskd).astype(ml_dtypes.bfloat16)

    ones_np = np.zeros((2, 128), np.float32)
    ones_np[0, :64] = 1.0
    ones_np[1, 64:] = 1.0
    ones_np = ones_np.astype(ml_dtypes.bfloat16)

    in_maps = []
    for c in range(NCORES):
        h0, h1 = HPC * c, HPC * c + 1
        # wh[p, g, a, c] = W_g[128a+p, c] with W_g = 2 heads side by side
        wh_c = np.stack(
            [
                np.concatenate([W_Q[h0], W_Q[h1]], axis=1),
                np.concatenate([W_K[h0], W_K[h1]], axis=1),
                np.concatenate([W_V[h0], W_V[h1]], axis=1),
            ]
        )  # [3, 1024, 128]
        wh_c = wh_c.reshape(3, NDC, 128, 128).transpose(2, 0, 1, 3)
        bqkv_c = np.stack(
            [
                np.concatenate([b_Q[h0], b_Q[h1]]),
                np.concatenate([b_K[h0], b_K[h1]]),
                np.concatenate([b_V[h0], b_V[h1]]),
            ],
            axis=1,
        ).astype(np.float32)
        wo_c = np.concatenate([W_O[h0], W_O[h1]], axis=0)
        in_maps.append(
            {
                "xh": xh,
                "wh": _to_bf16(wh_c),
                "bqkv": np.ascontiguousarray(bqkv_c),
                "wo": _to_bf16(wo_c),
                "maskd": maskd,
                "ones1": ones_np,
            }
        )
    return in_maps, b_O


def _gather(res, b_O):
    out = np.zeros((T, D), np.float32)
    for r in res.results:
        out += r["outp"].astype(np.float32)
    out += b_O[None, :]
    return out.reshape(B, S, D)


def kernel(
    normalized_resid_pre, W_Q, W_K, W_V, W_O, b_Q, b_K, b_V, b_O, **_unused
):
    in_maps, b_O = _prepare_in_maps(
        normalized_resid_pre, W_Q, W_K, W_V, W_O, b_Q, b_K, b_V, b_O
    )
    nc = _get_nc()
    res = run_bass_kernel_spmd(nc, in_maps, core_ids=list(range(NCORES)))
    return _gather(res, b_O)


def _try_install_profhook():
    """Register the axon NTFF profile hook (the container's antenv stub
    lacks axon_hooks); harmless no-op if anything is missing."""
    try:
        import sys
        import types

        if "antenv.axon_hooks" not in sys.modules:
            mod = types.ModuleType("antenv.axon_hooks")
            hook = [None]
            mod.set_axon_ntff_profile_hook = lambda h: hook.__setitem__(0, h)
            mod.get_axon_ntff_profile_hook = lambda: hook[0]
            sys.modules["antenv.axon_hooks"] = mod
            import antenv

            antenv.axon_hooks = mod
            from trn_agent_boot.trn_boot import _ntff_profile_via_ctypes

            mod.set_axon_ntff_profile_hook(
                _ntff_profile_via_ctypes("/opt/axon/libaxon_pjrt.so")
            )
            import concourse.bass_utils as bu

            bu.upload_artifacts = lambda tmpdir: f"file://{tmpdir}"
    except Exception:
        pass


def kernel_profiled(**inputs):
    """Like kernel() but with NTFF tracing; returns (out, BassKernelResults)."""
    _try_install_profhook()
    inputs = {k: v for k, v in inputs.items()}
    in_maps, b_O = _prepare_in_maps(
        inputs["normalized_resid_pre"],
        inputs["W_Q"],
        inputs["W_K"],
        inputs["W_V"],
        inputs["W_O"],
        inputs["b_Q"],
        inputs["b_K"],
        inputs["b_V"],
        inputs["b_O"],
    )
    nc = _get_nc()
    res = run_bass_kernel_spmd(
        nc, in_maps, core_ids=list(range(NCORES)), trace=True
    )
    return _gather(res, b_O), res


if __name__ == "__main__":
    rng = np.random.default_rng(0)
    inputs = {
        "normalized_resid_pre": rng.standard_normal((B, S, D)).astype(np.float32),
        "W_Q": (rng.standard_normal((NHEAD, D, HDIM)) * 0.02).astype(np.float32),
        "W_K": (rng.standard_normal((NHEAD, D, HDIM)) * 0.02).astype(np.float32),
        "W_V": (rng.standard_normal((NHEAD, D, HDIM)) * 0.02).astype(np.float32),
        "W_O": (rng.standard_normal((NHEAD, HDIM, D)) * 0.02).astype(np.float32),
        "b_Q": np.zeros((NHEAD, HDIM), np.float32),
        "b_K": np.zeros((NHEAD, HDIM), np.float32),
        "b_V": np.zeros((NHEAD, HDIM), np.float32),
        "b_O": np.zeros((D,), np.float32),
    }
    out = kernel(**inputs)
    print("out", out.shape, out.dtype, float(np.abs(out).max()))


# revision 17
# speedup vs baseline: 1.2848x; 1.0621x over previous
"""Causal multi-head attention (B=2, S=2048, D=1024, 16 heads x 64) on 8
Trainium2 NeuronCores.

Sharding: tensor-parallel over heads — 2 heads per core. Each core gets the
full (pre-transposed, bf16-cast) activations and its 2 heads' weights,
computes q/k/v projections, causal flash-style attention, and a partial
output projection; the host sums the 8 partial outputs and adds b_O.

Device algorithm per core (all matmuls bf16 with fp32 PSUM accumulate):
  - QKV:   qT/kT/vT [128=2*64 headdims, 4096 tok] = W.T @ xT, accumulated
           over 8 contraction chunks of 128. Host pre-arranges x and W into
           partition-major chunk-contiguous DRAM layouts so every DMA is a
           full-rate contiguous burst; late constants ride the gpsimd DMA
           queue so the sync queue only carries the critical x/w stream.
  - V is re-laid-out to [pos, headdim] via DVE 32x32 block transposes, with
    a ones-column appended so the attention-value matmul also produces the
    softmax denominator for free.
  - scores are computed transposed (key position on partitions) so softmax's
    sum folds into the AV matmul; the two heads' score matmuls run
    concurrently in disjoint PE row-groups. exp runs on the scalar engine
    straight out of PSUM. On diagonal key-tiles everything (scores, exp,
    mask, AV) is trimmed to the causally-needed query range; the causal
    mask multiply reduces to one shared 128x128 lower-triangle block.
  - QKV phase is interleaved with attention units per 1024-token block so
    the scalar engine's exp stream starts ~12us in instead of ~60us.
  - Q/K bias-add+cast run on the vector engine (tensor_scalar_add), V on
    the scalar engine: balances ACT vs DVE across the whole kernel.
  - 1/sum computed as exp(-ln(sum)) on the scalar engine, broadcast across
    partitions with a rank-2 matmul, applied while evacuating z.
  - out projection contracts both heads (128 partitions) in one matmul;
    its PSUM evacuation alternates engines at the tail.
"""

import functools

import numpy as np
import ml_dtypes

import concourse.bass as bass
import concourse.tile as tile
import concourse.mybir as mybir
from concourse.bass_utils import run_bass_kernel_spmd

# ---------------------------------------------------------------- wait fix
# This container's walrus accepts at most ONE sync-wait per instruction
# (two for EventSemaphore); Tile emits several. Hoist the excess onto NoOps
# inserted just before the over-subscribed instruction on the same engine.
import json as _json

_WAIT_CAP = {"EventSemaphore": 2}


def _split_waits(doc):
    n = [0]

    def fix_block(block):
        insts = block.get("instructions")
        if not isinstance(insts, list):
            return
        out = []
        for inst in insts:
            si = inst.get("sync_info")
            waits = si.get("on_wait") if si else None
            cap = _WAIT_CAP.get(inst.get("opcode"), 1)
            if waits and len(waits) > cap:
                for w in waits[cap:]:
                    n[0] += 1
                    out.append(
                        {
                            "name": f"WSPL-{n[0]}",
                            "opcode": "NoOp",
                            "engine": inst["engine"],
                            "ins": [],
                            "outs": [],
                            "sync_info": {"on_wait": [w], "on_update": []},
                        }
                    )
                si["on_wait"] = waits[:cap]
            out.append(inst)
        block["instructions"] = out

    def walk(o):
        if isinstance(o, dict):
            if "instructions" in o:
                fix_block(o)
            for v in o.values():
                walk(v)
        elif isinstance(o, list):
            for v in o:
                walk(v)

    walk(doc)
    return doc


_waitfix_done = False


def _install_waitfix():
    global _waitfix_done
    if _waitfix_done:
        return
    _waitfix_done = True
    orig = bass.Bass.to_json_bytes

    def to_json_bytes(self, *a, **kw):
        doc = _json.loads(orig(self, *a, **kw))
        return _json.dumps(_split_waits(doc)).encode()

    bass.Bass.to_json_bytes = to_json_bytes


# ---------------------------------------------------------------- constants
B, S, D = 2, 2048, 1024
NHEAD, HDIM = 16, 64
T = B * S  # 4096 tokens
NCORES = 8
HPC = NHEAD // NCORES  # 2 heads per core
SCALE = 1.0 / 8.0  # 1/sqrt(HDIM)

bf16 = mybir.dt.bfloat16
f32 = mybir.dt.float32
AF = mybir.ActivationFunctionType

NDC = D // 128  # 8 contraction chunks
NCHUNK = T // 512  # 8 token chunks of 512
NKT = S // 128  # 16 key tiles per batch
NQB = S // 512  # 4 query blocks per batch


def _build_nc():
    nc = bass.Bass()
    # chunk-major x: xh[p, k, a, m] = x[512k+m, 128a+p] — each 512-token
    # chunk is one contiguous 8KB-per-partition DMA into the identically
    # laid-out SBUF tile
    xh = nc.dram_tensor("xh", [128, NCHUNK, NDC, 512], bf16, kind="ExternalInput")
    # group-major qkv weights: wh[p, g, a, c] = W_g[128a+p, c]
    wh = nc.dram_tensor("wh", [128, 3, NDC, 128], bf16, kind="ExternalInput")
    bqkv = nc.dram_tensor("bqkv", [128, 3], f32, kind="ExternalInput")
    wo = nc.dram_tensor("wo", [128, D], bf16, kind="ExternalInput")
    # single lower-triangle mask block (kk <= qq), replicated for 2 heads
    maskd = nc.dram_tensor("maskd", [128, HPC, 128], bf16, kind="ExternalInput")
    ones1 = nc.dram_tensor("ones1", [2, 128], bf16, kind="ExternalInput")
    outp = nc.dram_tensor("outp", [T, D], bf16, kind="ExternalOutput")

    with tile.TileContext(nc) as tc:
        with (
            tc.tile_pool(name="const", bufs=1) as const,
            tc.tile_pool(name="attn", bufs=8) as attnp,
            tc.tile_pool(name="obuf", bufs=4) as obufp,
            tc.tile_pool(name="small", bufs=4) as small,
            tc.tile_pool(name="psum", bufs=2, space="PSUM") as psum,
        ):
            # ---- constant loads. sync queue: V weights, then x chunks in
            # consumption order (chunk 0 split in half so the first matmul
            # group can start ~1.5us earlier), then Q/K weights.
            # gpsimd queue (idle engine): bias, ones, mask, wo.
            w_sb = const.tile([128, 3, NDC, 128], bf16)
            xt_sb = const.tile([128, NCHUNK, NDC, 512], bf16)
            nc.sync.dma_start(w_sb[:, 2], wh[:, 2])
            nc.sync.dma_start(xt_sb[:, 0, :, 0:256], xh[:, 0, :, 0:256])
            nc.sync.dma_start(xt_sb[:, 0, :, 256:512], xh[:, 0, :, 256:512])
            nc.sync.dma_start(xt_sb[:, 1], xh[:, 1])
            nc.sync.dma_start(w_sb[:, 0], wh[:, 0])
            nc.sync.dma_start(w_sb[:, 1], wh[:, 1])

            # bias gates every projection cast: issue it on the scalar
            # queue (the scalar engine is idle for the first ~10us, and
            # the 12-byte-run descriptor generation would stall the sync
            # queue's x-chunk stream)
            bias_sb = const.tile([128, 3], f32)
            nc.scalar.dma_start(bias_sb[:], bqkv[:])
            ee_sb = const.tile([2, 128], bf16)
            nc.scalar.dma_start(ee_sb[:], ones1[:])
            mask_sb = const.tile([128, HPC, 128], bf16)
            nc.gpsimd.dma_start(mask_sb[:], maskd[:])
            wo_sb = const.tile([128, D], bf16)
            nc.gpsimd.dma_start(wo_sb[:], wo[:])

            for k in range(2, NCHUNK):
                nc.sync.dma_start(xt_sb[:, k], xh[:, k])

            qT = const.tile([128, T], bf16)
            kT = const.tile([128, T], bf16)
            vT = const.tile([128, T], bf16)
            zT = const.tile([128, T], bf16)
            qkvT = (qT, kT, vT)

            v_sb = []
            for h in range(HPC):
                v = const.tile([128, T // 128, 65], bf16, name=f"v_sb{h}")
                nc.gpsimd.memset(v[:, :, 64], 1.0)
                v_sb.append(v)

            # ---- QKV decomposed into granules so it can be sprinkled into
            # attention units' key-tile slots: the in-order PE fills its
            # exp-wait gaps with projection matmuls and the scalar engine's
            # exp stream never drains while a bulk QKV block runs.
            #   m_granule(pp, half, g): one 512-token projection group
            #     (8 accumulating matmuls + bias-cast; V on ACT, Q/K on DVE)
            #   t_granule(pp, h): one head's V re-layout for a 1024-token
            #     block (8 DVE 32x32 block-transposes)
            vt4 = vT[:].rearrange("p (t x i) -> p t x i", x=4, i=32)

            def m_granule(pp, half, g):
                def go():
                    pt = 2 * pp + half
                    ps = psum.tile([128, 512], f32, tag="sc", bufs=3)
                    for di in range(NDC):
                        nc.tensor.matmul(
                            ps[:],
                            w_sb[:, g, di, :],
                            xt_sb[:, pt, di, :],
                            start=(di == 0),
                            stop=(di == NDC - 1),
                        )
                    dst = qkvT[g][:, 512 * pt : 512 * pt + 512]
                    if g == 2:
                        nc.scalar.activation(
                            dst,
                            ps[:],
                            AF.Identity,
                            bias=bias_sb[:, g : g + 1],
                            scale=1.0,
                        )
                    else:
                        nc.vector.tensor_scalar_add(dst, ps[:], bias_sb[:, g : g + 1])

                return go

            def t_granule(pp, h):
                def go():
                    ts = slice(8 * pp, 8 * pp + 8)
                    for al in range(4):
                        for bb in range(2):
                            nc.vector.transpose(
                                v_sb[h][
                                    32 * al : 32 * al + 32, ts, 32 * bb : 32 * bb + 32
                                ],
                                vt4[
                                    64 * h + 32 * bb : 64 * h + 32 * bb + 32, ts, al, :
                                ],
                            )

                return go

            # ---- attention per (batch, 512-query-block). Each block's
            # output projection is emitted two units LATE so the in-order
            # PE always has ready matmuls while the previous block's
            # normalize chain completes.
            def emit_outproj_tile(qb, b, qx, tail=False):
                qt = NKT * b + 4 * qb + qx
                op = psum.tile([128, 1024], f32, tag="sc", bufs=3, name="op")
                for dh in range(2):
                    nc.tensor.matmul(
                        op[:, 512 * dh : 512 * dh + 512],
                        zT[:, 128 * qt : 128 * qt + 128],
                        wo_sb[:, 512 * dh : 512 * dh + 512],
                        start=True,
                        stop=True,
                    )
                ob = obufp.tile([128, 1024], bf16, name="ob")
                if tail and qx % 2 == 1:
                    # drain: no exps left, the idle scalar engine
                    # shares the PSUM evacuation load with DVE
                    nc.scalar.copy(ob[:], op[:])
                else:
                    nc.vector.tensor_copy(ob[:], op[:])
                nc.sync.dma_start(outp[128 * qt : 128 * qt + 128, :], ob[:])

            def emit_outproj(qb, b, tail=False):
                for qx in range(4):
                    emit_outproj_tile(qb, b, qx, tail=tail)

            def norm_stage_a(st_):
                # 1/sum = exp(-ln(sum)); both heads' sums were DMA-staged
                # onto partitions {0,1} of one tile, so one ln and one exp
                # cover both heads
                q0, zsU, rsin, rs2 = st_
                lnS = small.tile([2, 512], f32, tag="lnS")
                nc.scalar.activation(lnS[:], rsin[:], AF.Ln, scale=1.0)
                nc.scalar.activation(rs2[:], lnS[:], AF.Exp, scale=-1.0)

            def norm_stage_b(st_):
                # broadcast both heads' reciprocals to 128 partitions with
                # one K=2 matmul against the 0/1 selector matrix ee_sb,
                # then normalize the staged z into zT
                q0, zsU, rsin, rs2 = st_
                rbP = psum.tile([128, 512], f32, tag="sc", bufs=3, name="rbP")
                nc.tensor.matmul(rbP[:], ee_sb[:], rs2[:], start=True, stop=True)
                for h in range(HPC):
                    nc.vector.tensor_mul(
                        zT[64 * h : 64 * h + 64, q0 : q0 + 512],
                        zsU[h][0:64, :],
                        rbP[64 * h : 64 * h + 64, :],
                    )

            st = {"norm_a": None, "norm_b": None, "uidx": 0}
            out_queue = []  # (uidx, qb, b); emitted two units late

            def emit_unit(qb, b, granules=None, last=False):
                uidx = st["uidx"]
                st["uidx"] += 1
                norm_a = st["norm_a"]
                norm_b = st["norm_b"]
                nkt = 4 * (qb + 1)  # causal: key tiles 0..4qb+3
                q0 = S * b + 512 * qb
                granules = dict(granules or {})
                zp = [
                    psum.tile([65, 512], f32, tag="z", bufs=2, name=f"zp{h}")
                    for h in range(HPC)
                ]
                # pop deferred output-projection blocks (two at the final
                # unit so the tail only carries its own) and spread the
                # tiles across this unit's key-tile iterations so each op
                # PSUM tile is evacuated before the next is written
                op_todo = []
                lag = 1 if last else 2
                op_slots = (
                    [1, 2, 3, 3]
                    if nkt == 4
                    else [2, 3, 5, 6, 9, 10, 11, 12, 13, 14, 15, 15]
                )
                while out_queue and out_queue[0][0] <= uidx - lag:
                    _, oqb, ob_ = out_queue.pop(0)
                    for qx in range(4):
                        op_todo.append((op_slots.pop(0), oqb, ob_, qx))

                # software pipeline: AV(kt) is emitted during iteration
                # kt+1, so the in-order PE streams scores(kt+1) while the
                # scalar engine runs exp(kt); AV never stalls the chain.
                pend = None  # (at, gk, trim, is_last)

                def emit_av(p):
                    at_, gk_, trim_, last_ = p
                    for h in range(HPC):
                        nc.tensor.matmul(
                            zp[h][:, trim_:512],
                            v_sb[h][:, gk_, :],
                            at_[:, 512 * h + trim_ : 512 * h + 512],
                            start=(gk_ % NKT == 0),
                            stop=last_,
                            skip_group_check=True,
                        )

                for kt in range(nkt):
                    gk = NKT * b + kt
                    j = kt - 4 * qb  # >=0 on diagonal key-tiles
                    trim = 128 * j if j >= 0 else 0
                    # flat [128, 1024] tiles: a 2D AP is ~200ns/instruction
                    # cheaper on ACT than the equivalent 3D view; 3D views
                    # are used only for the trimmed diagonal slices
                    sp = psum.tile([128, 1024], f32, tag="sc", bufs=3)
                    for h in range(HPC):
                        nc.tensor.matmul(
                            sp[:, 512 * h + trim : 512 * h + 512],
                            kT[64 * h : 64 * h + 64, 128 * gk : 128 * gk + 128],
                            qT[64 * h : 64 * h + 64, q0 + trim : q0 + 512],
                            start=True,
                            stop=True,
                        )
                    at = attnp.tile([128, 1024], bf16)
                    if trim > 0:
                        sp3 = sp[:].rearrange("p (h q) -> p h q", h=2)
                        at3 = at[:].rearrange("p (h q) -> p h q", h=2)
                        nc.scalar.activation(
                            at3[:, :, trim:512],
                            sp3[:, :, trim:512],
                            AF.Exp,
                            scale=SCALE,
                        )
                    else:
                        nc.scalar.activation(at[:], sp[:], AF.Exp, scale=SCALE)
                    if j >= 0:
                        # causal mask: only the 128-wide diagonal block of
                        # the trimmed range can contain masked entries
                        at3 = at[:].rearrange("p (h q) -> p h q", h=2)
                        nc.vector.tensor_mul(
                            at3[:, :, trim : trim + 128],
                            at3[:, :, trim : trim + 128],
                            mask_sb[:],
                        )
                    g = granules.pop(kt, None)
                    if g is not None:
                        g()
                    if pend is not None:
                        emit_av(pend)
                    pend = (at, gk, trim, kt == nkt - 1)
                    if kt == 1 and norm_a is not None:
                        norm_stage_a(norm_a)
                        norm_b = norm_a
                        norm_a = None
                    if kt == min(4, nkt - 2) and norm_b is not None:
                        norm_stage_b(norm_b)
                        norm_b = None
                    while op_todo and op_todo[0][0] == kt:
                        _, oqb, ob_, qx = op_todo.pop(0)
                        emit_outproj_tile(oqb, ob_, qx)
                emit_av(pend)
                assert not granules, f"unplaced granules {granules.keys()}"
                # evacuate z and its sums row to SBUF immediately so the
                # PSUM banks free up for the next query block; a small DMA
                # gathers the two sums rows onto partitions {0,1} of one
                # tile (DMA writes have no partition-alignment limits)
                zsU = [
                    small.tile([65, 512], bf16, tag=f"zsU{h}", name=f"zsU{h}")
                    for h in range(HPC)
                ]
                rsin = small.tile([2, 512], bf16, tag="rsin")
                rs2 = small.tile([2, 512], bf16, tag="rs2")
                for h in range(HPC):
                    nc.vector.tensor_copy(zsU[h][:], zp[h][:])
                    nc.sync.dma_start(rsin[h : h + 1, :], zsU[h][64:65, :])
                st["norm_a"] = (q0, zsU, rsin, rs2)
                st["norm_b"] = norm_b
                out_queue.append((uidx, qb, b))

            # ---- master schedule. A minimal prelude computes q/k/v for
            # tokens 0-511 (plus V of 512-1023 so the V re-layout covers
            # the first block); every remaining projection granule is
            # placed inside an attention unit, at key-tile slots not used
            # by norm/outproj PSUM tiles, always before its first
            # consumer. After the prelude the scalar engine's exp stream
            # runs continuously to the end of the kernel.
            m, t = m_granule, t_granule
            for g in (m(0, 0, 2), m(0, 1, 2), m(0, 0, 0), m(0, 0, 1),
                      t(0, 0), t(0, 1)):
                g()
            emit_unit(0, 0, {0: m(0, 1, 0), 1: m(0, 1, 1)})
            emit_unit(1, 0, {0: m(1, 0, 2), 2: m(1, 1, 2), 3: m(1, 0, 0),
                             5: m(1, 0, 1), 6: t(1, 0), 7: t(1, 1)})
            emit_unit(2, 0, {0: m(1, 1, 0), 1: m(1, 1, 1), 7: m(2, 0, 2),
                             8: m(2, 1, 2), 9: m(2, 0, 0), 10: m(2, 0, 1),
                             11: t(2, 0)})
            emit_unit(3, 0, {0: t(2, 1), 1: m(2, 1, 0), 7: m(2, 1, 1),
                             8: m(3, 0, 2), 9: m(3, 1, 2), 10: m(3, 0, 0),
                             11: m(3, 0, 1), 12: t(3, 0), 13: t(3, 1)})
            emit_unit(0, 1)
            emit_unit(1, 1, {0: m(3, 1, 0), 1: m(3, 1, 1)})
            emit_unit(2, 1)
            emit_unit(3, 1, last=True)

            # tail: only the final unit's own normalize + output
            # projection remain
            norm_stage_a(st["norm_a"])
            norm_stage_b(st["norm_a"])
            assert len(out_queue) == 1
            _, oqb, ob_ = out_queue.pop(0)
            emit_outproj(oqb, ob_, tail=True)

    return nc


@functools.lru_cache(maxsize=1)
def _get_nc():
    _install_waitfix()
    return _build_nc()


def _to_bf16(a):
    return np.ascontiguousarray(np.asarray(a, dtype=np.float32)).astype(
        ml_dtypes.bfloat16
    )


def _prepare_in_maps(
    normalized_resid_pre, W_Q, W_K, W_V, W_O, b_Q, b_K, b_V, b_O
):
    x = np.asarray(normalized_resid_pre, dtype=np.float32)
    W_Q = np.asarray(W_Q, dtype=np.float32)
    W_K = np.asarray(W_K, dtype=np.float32)
    W_V = np.asarray(W_V, dtype=np.float32)
    W_O = np.asarray(W_O, dtype=np.float32)
    b_Q = np.asarray(b_Q, dtype=np.float32)
    b_K = np.asarray(b_K, dtype=np.float32)
    b_V = np.asarray(b_V, dtype=np.float32)
    b_O = np.asarray(b_O, dtype=np.float32)

    # xh[p, k, a, m] = x[512k+m, 128a+p]
    xh = _to_bf16(
        x.reshape(T, D).reshape(NCHUNK, 512, NDC, 128).transpose(3, 0, 2, 1)
    )

    # shared lower-triangle diagonal mask block (kk <= qq), both heads
    kk = np.arange(128)[:, None]
    qq = np.arange(128)[None, :]
    maskd = np.broadcast_to(
        (kk <= qq).astype(np.float32)[:, None, :], (128, HPC, 128)
    )
    maskd = np.ascontiguousarray(maskd).astype(ml_dtypes.bfloat16)

    ones_np = np.zeros((2, 128), np.float32)
    ones_np[0, :64] = 1.0
    ones_np[1, 64:] = 1.0
    ones_np = ones_np.astype(ml_dtypes.bfloat16)

    in_maps = []
    for c in range(NCORES):
        h0, h1 = HPC * c, HPC * c + 1
        # wh[p, g, a, c] = W_g[128a+p, c] with W_g = 2 heads side by side
        wh_c = np.stack(
            [
                np.concatenate([W_Q[h0], W_Q[h1]], axis=1),
                np.concatenate([W_K[h0], W_K[h1]], axis=1),
                np.concatenate([W_V[h0], W_V[h1]], axis=1),
            ]
        )  # [3, 1024, 128]
        wh_c = wh_c.reshape(3, NDC, 128, 128).transpose(2, 0, 1, 3)
        bqkv_c = np.stack(
            [
                np.concatenate([b_Q[h0], b_Q[h1]]),
                np.concatenate([b_K[h0], b_K[h1]]),
                np.concatenate([b_V[h0], b_V[h1]]),
            ],
            axis=1,
        ).astype(np.float32)
        wo_c = np.concatenate([W_O[h0], W_O[h1]], axis=0)
        in_maps.append(
            {
                "xh": xh,
                "wh": _to_bf16(wh_c),
                "bqkv": np.ascontiguousarray(bqkv_c),
                "wo": _to_bf16(wo_c),
                "maskd": maskd,
                "ones1": ones_np,
            }
        )
    return in_maps, b_O


def _gather(res, b_O):
    out = np.zeros((T, D), np.float32)
    for r in res.results:
        out += r["outp"].astype(np.float32)
    out += b_O[None, :]
    return out.reshape(B, S, D)


def kernel(
    normalized_resid_pre, W_Q, W_K, W_V, W_O, b_Q, b_K, b_V, b_O, **_unused
):
    in_maps, b_O = _prepare_in_maps(
        normalized_resid_pre, W_Q, W_K, W_V, W_O, b_Q, b_K, b_V, b_O
    )
    nc = _get_nc()
    res = run_bass_kernel_spmd(nc, in_maps, core_ids=list(range(NCORES)))
    return _gather(res, b_O)


def _try_install_profhook():
    """Register the axon NTFF profile hook (the container's antenv stub
    lacks axon_hooks); harmless no-op if anything is missing."""
    try:
        import sys
        import types

        if "antenv.axon_hooks" not in sys.modules:
            mod = types.ModuleType("antenv.axon_hooks")
            hook = [None]
            mod.set_axon_ntff_profile_hook = lambda h: hook.__setitem__(0, h)
            mod.get_axon_ntff_profile_hook = lambda: hook[0]
            sys.modules["antenv.axon_hooks"] = mod
            import antenv

            antenv.axon_hooks = mod
            from trn_agent_boot.trn_boot import _ntff_profile_via_ctypes

            mod.set_axon_ntff_profile_hook(
                _ntff_profile_via_ctypes("/opt/axon/libaxon_pjrt.so")
            )
            import concourse.bass_utils as bu

            bu.upload_artifacts = lambda tmpdir: f"file://{tmpdir}"
    except Exception:
        pass


def kernel_profiled(**inputs):
    """Like kernel() but with NTFF tracing; returns (out, BassKernelResults)."""
    _try_install_profhook()
    inputs = {k: v for k, v in inputs.items()}
    in_maps, b_O = _prepare_in_maps(
        inputs["normalized_resid_pre"],
        inputs["W_Q"],
        inputs["W_K"],
        inputs["W_V"],
        inputs["W_O"],
        inputs["b_Q"],
        inputs["b_K"],
        inputs["b_V"],
        inputs["b_O"],
    )
    nc = _get_nc()
    res = run_bass_kernel_spmd(
        nc, in_maps, core_ids=list(range(NCORES)), trace=True
    )
    return _gather(res, b_O), res


if __name__ == "__main__":
    rng = np.random.default_rng(0)
    inputs = {
        "normalized_resid_pre": rng.standard_normal((B, S, D)).astype(np.float32),
        "W_Q": (rng.standard_normal((NHEAD, D, HDIM)) * 0.02).astype(np.float32),
        "W_K": (rng.standard_normal((NHEAD, D, HDIM)) * 0.02).astype(np.float32),
        "W_V": (rng.standard_normal((NHEAD, D, HDIM)) * 0.02).astype(np.float32),
        "W_O": (rng.standard_normal((NHEAD, HDIM, D)) * 0.02).astype(np.float32),
        "b_Q": np.zeros((NHEAD, HDIM), np.float32),
        "b_K": np.zeros((NHEAD, HDIM), np.float32),
        "b_V": np.zeros((NHEAD, HDIM), np.float32),
        "b_O": np.zeros((D,), np.float32),
    }
    out = kernel(**inputs)
    print("out", out.shape, out.dtype, float(np.abs(out).max()))
